# revision 1
# baseline (speedup 1.0000x reference)
"""CrissCrossAttention Trainium2 kernel.

Full inputs -> shard batch over 8 NeuronCores (2 batches/core) -> SPMD Bass/Tile
kernel -> gather full output.

Per-core math (B_local=2, C=2048, n=H*W=1024, heads=2, d=C/heads=1024==n):
  qkv   = W_qkv @ X            (per batch, [3C, n])
  per head: E_h = Q^T K  -> A_h = softmax rows -> O_h = V A_h^T
            E_v = Q K^T  -> A_v = softmax rows -> O_v = A_v V^T
  Y = gamma * (W_out @ (O_h + O_v)) + X

TensorE runs everything in float32r (full-rate at N>=256, ~tf32 precision).
Weights are transposed (and gamma folded into w_out) on the host, so the
stationary operands stream straight from DRAM.
"""

import numpy as np

import concourse.bass as bass
import concourse.mybir as mybir
import concourse.tile as tile
from concourse import bacc
from concourse.bass_utils import run_bass_kernel_spmd
from concourse.masks import make_identity

F32 = mybir.dt.float32
F32R = mybir.dt.float32r
BF16 = mybir.dt.bfloat16
AX = mybir.AxisListType.X
EXP = mybir.ActivationFunctionType.Exp
NCORES = 8


def build_kernel(Bl, C, n, heads):
    d = C // heads
    assert d == n, "module requires H*W == C//heads"
    O3 = 3 * C
    cch = C // 128           # c-chunks (contraction tiles for conv/proj)
    dch = d // 128           # d-chunks per head
    nch = n // 128           # n-chunks
    NHALF = min(512, n)
    nh2 = n // NHALF         # output column halves
    VW = 256                 # v-proj rhs chunk width
    hc = cch // 2

    nc = bacc.Bacc("TRN2", target_bir_lowering=False)

    x_in = nc.declare_dram_parameter("x", [Bl, C, n], F32R, isOutput=False)
    wqkvT = nc.declare_dram_parameter("wqkvT", [C, O3], F32R, isOutput=False)
    woutT = nc.declare_dram_parameter("woutT", [C, C], F32R, isOutput=False)
    y_out = nc.declare_dram_parameter("y", [Bl, C, n], F32, isOutput=True)

    with tile.TileContext(nc) as tc:
        with tc.tile_pool(name="big", bufs=1) as big, \
             tc.tile_pool(name="wp", bufs=2) as wp, \
             tc.tile_pool(name="eb", bufs=2) as eb, \
             tc.tile_pool(name="stp", bufs=3) as stp, \
             tc.tile_pool(name="smp", bufs=16) as smp, \
             tc.tile_pool(name="one", bufs=1) as one, \
             tc.tile_pool(name="dr", bufs=1, space="DRAM") as dr, \
             tc.tile_pool(name="psA", bufs=4, space="PSUM") as psA, \
             tc.tile_pool(name="psT", bufs=4, space="PSUM") as psT:

            qbuf = dr.tile([Bl, C, n], F32R, tag="qbuf")
            kbuf = dr.tile([Bl, C, n], F32R, tag="kbuf")
            vtbuf = dr.tile([Bl, n, C], BF16, tag="vtbuf")
            obuf = dr.tile([Bl, C, n], F32R, tag="obuf")

            ident = one.tile([128, 128], F32, tag="ident")
            make_identity(nc, ident)
            idr = one.tile([128, 128], F32R, tag="identr")
            nc.vector.tensor_copy(idr, ident)

            def proj(b):
                """qkv projection for batch b: writes qbuf/kbuf (natural
                [d, n]) and vtbuf (transposed [n, d_v])."""
                x3a = big.tile([128, hc, n], F32R, tag="bigA")
                x3b = big.tile([128, cch - hc, n], F32R, tag="bigB")
                nc.sync.dma_start(
                    out=x3a,
                    in_=x_in[b, 0:hc * 128].rearrange("(ci p) n -> p ci n", p=128))
                nc.sync.dma_start(
                    out=x3b,
                    in_=x_in[b, hc * 128:].rearrange("(ci p) n -> p ci n", p=128))

                def xci(ci):
                    return x3a[:, ci] if ci < hc else x3b[:, ci - hc]

                # Q, K natural orientation: out[o-tile, n] = W^T.T @ X
                for ot in range(2 * cch):
                    wt = wp.tile([128, cch, 128], F32R, tag="w")
                    nc.sync.dma_start(
                        out=wt,
                        in_=wqkvT[:, ot * 128:(ot + 1) * 128]
                        .rearrange("(ci p) o -> p ci o", p=128))
                    for nh in range(nh2):
                        acc = psA.tile([128, NHALF], F32, tag="acc")
                        for ci in range(cch):
                            nc.tensor.matmul(
                                acc, wt[:, ci],
                                xci(ci)[:, nh * NHALF:(nh + 1) * NHALF],
                                start=(ci == 0), stop=(ci == cch - 1))
                        st = stp.tile([128, NHALF], F32R, tag="st")
                        nc.scalar.copy(st, acc)
                        if ot < cch:
                            dst = qbuf[b, ot * 128:(ot + 1) * 128]
                        else:
                            dst = kbuf[b, (ot - cch) * 128:(ot - cch + 1) * 128]
                        nc.sync.dma_start(
                            out=dst[:, nh * NHALF:(nh + 1) * NHALF], in_=st)

                # V transposed: out[n-tile, o_v] = X.T @ W^T  (X stationary)
                for vh in range(C // VW):
                    wv = eb.tile([128, cch, VW], F32R, tag="ebk")
                    nc.sync.dma_start(
                        out=wv,
                        in_=wqkvT[:, 2 * C + vh * VW:2 * C + (vh + 1) * VW]
                        .rearrange("(ci p) o -> p ci o", p=128))
                    for nt in range(nch):
                        acc = psA.tile([128, VW], F32, tag="acc")
                        for ci in range(cch):
                            nc.tensor.matmul(
                                acc, xci(ci)[:, nt * 128:(nt + 1) * 128],
                                wv[:, ci],
                                start=(ci == 0), stop=(ci == cch - 1))
                        st = stp.tile([128, VW], BF16, tag="st")
                        nc.scalar.copy(st, acc)
                        nc.sync.dma_start(
                            out=vtbuf[b, nt * 128:(nt + 1) * 128,
                                      vh * VW:(vh + 1) * VW], in_=st)

            def softmax_rowtile(accs, dst_row):
                """softmax over the free axis of a [128, n] row tile held in
                nh2 PSUM halves; writes normalized rows to dst_row [128, n]."""
                negs = []
                for mh in range(nh2):
                    nm = smp.tile([128, 1], F32, tag="sc")
                    nc.vector.reduce_max(nm, accs[mh], axis=AX, negate=True)
                    negs.append(nm)
                nm = negs[0]
                for mh in range(1, nh2):
                    nm2 = smp.tile([128, 1], F32, tag="sc")
                    nc.vector.tensor_tensor(
                        out=nm2, in0=nm, in1=negs[mh], op=mybir.AluOpType.min)
                    nm = nm2
                sums = []
                for mh in range(nh2):
                    s = smp.tile([128, 1], F32, tag="sc")
                    nc.scalar.activation(
                        dst_row[:, mh * NHALF:(mh + 1) * NHALF], accs[mh],
                        EXP, bias=nm, scale=1.0, accum_out=s)
                    sums.append(s)
                stot = sums[0]
                for mh in range(1, nh2):
                    s2 = smp.tile([128, 1], F32, tag="sc")
                    nc.vector.tensor_tensor(
                        out=s2, in0=stot, in1=sums[mh], op=mybir.AluOpType.add)
                    stot = s2
                r = smp.tile([128, 1], F32, tag="sc")
                nc.vector.reciprocal(r, stot)
                nc.vector.tensor_scalar_mul(dst_row, dst_row, r)

            def transpose_into(src128, dst3, nj_dst, col_dst, dt):
                """PE-transpose one [128,128] block into dst3[:, nj_dst,
                col_dst*128:...] via a PSUM bounce."""
                pt = psT.tile([128, 128], dt, tag="tr")
                nc.tensor.transpose(pt, src128, idr if dt == F32R else ident)
                nc.scalar.copy(dst3[:, nj_dst, col_dst * 128:(col_dst + 1) * 128], pt)

            def attn(b, h):
                q3 = big.tile([128, dch, n], F32R, tag="bigA")
                k3 = big.tile([128, dch, n], F32R, tag="bigB")
                nc.sync.dma_start(
                    out=q3, in_=qbuf[b, h * d:(h + 1) * d]
                    .rearrange("(ci p) n -> p ci n", p=128))
                nc.sync.dma_start(
                    out=k3, in_=kbuf[b, h * d:(h + 1) * d]
                    .rearrange("(ci p) n -> p ci n", p=128))

                qt3 = big.tile([128, nch, d], F32R, tag="bigC")
                kt3 = big.tile([128, nch, d], F32R, tag="bigD")
                aht3 = big.tile([128, nch, n], BF16, tag="bigF")

                # E_h = Q^T K, row-softmax, transpose A_h into aht3
                for jb in range(nch // 2):
                    ab = eb.tile([128, 2, n], F32, tag="ebk")
                    for jj in range(2):
                        jt = jb * 2 + jj
                        accs = []
                        for mh in range(nh2):
                            acc = psA.tile([128, NHALF], F32, tag="acc")
                            for ci in range(dch):
                                nc.tensor.matmul(
                                    acc, q3[:, ci, jt * 128:(jt + 1) * 128],
                                    k3[:, ci, mh * NHALF:(mh + 1) * NHALF],
                                    start=(ci == 0), stop=(ci == dch - 1))
                            accs.append(acc)
                        softmax_rowtile(accs, ab[:, jj])
                        for mi in range(nch):
                            transpose_into(
                                ab[:, jj, mi * 128:(mi + 1) * 128],
                                aht3, mi, jt, F32)

                # transposes of Q and K (after E_h reads complete)
                for ci in range(dch):
                    for nj in range(nch):
                        transpose_into(
                            q3[:, ci, nj * 128:(nj + 1) * 128], qt3, nj, ci, F32R)
                        transpose_into(
                            k3[:, ci, nj * 128:(nj + 1) * 128], kt3, nj, ci, F32R)

                # E_v = Q K^T from transposed operands; A_v^T into avt3 (slab A)
                avt3 = big.tile([128, dch, d], BF16, tag="bigA")
                vt3 = big.tile([128, nch, d], BF16, tag="bigB")
                nc.sync.dma_start(
                    out=vt3, in_=vtbuf[b, :, h * d:(h + 1) * d]
                    .rearrange("(mi p) dd -> p mi dd", p=128))
                for ib in range(dch // 2):
                    ab = eb.tile([128, 2, d], F32, tag="ebk")
                    for jj in range(2):
                        it = ib * 2 + jj
                        accs = []
                        for eh in range(nh2):
                            acc = psA.tile([128, NHALF], F32, tag="acc")
                            for mi in range(nch):
                                nc.tensor.matmul(
                                    acc, qt3[:, mi, it * 128:(it + 1) * 128],
                                    kt3[:, mi, eh * NHALF:(eh + 1) * NHALF],
                                    start=(mi == 0), stop=(mi == nch - 1))
                            accs.append(acc)
                        softmax_rowtile(accs, ab[:, jj])
                        for ei in range(dch):
                            transpose_into(
                                ab[:, jj, ei * 128:(ei + 1) * 128],
                                avt3, ei, it, F32)

                # O = V A_h^T + A_v V^T accumulated in one PSUM group
                for it in range(dch):
                    for jh in range(nh2):
                        acc = psA.tile([128, NHALF], F32, tag="acc")
                        for mi in range(nch):
                            nc.tensor.matmul(
                                acc, vt3[:, mi, it * 128:(it + 1) * 128],
                                aht3[:, mi, jh * NHALF:(jh + 1) * NHALF],
                                start=(mi == 0), stop=False)
                        for ei in range(dch):
                            nc.tensor.matmul(
                                acc, avt3[:, ei, it * 128:(it + 1) * 128],
                                vt3[:, ei, jh * NHALF:(jh + 1) * NHALF],
                                start=False, stop=(ei == dch - 1))
                        st = stp.tile([128, NHALF], F32R, tag="st")
                        nc.scalar.copy(st, acc)
                        nc.sync.dma_start(
                            out=obuf[b, h * d + it * 128:h * d + (it + 1) * 128,
                                     jh * NHALF:(jh + 1) * NHALF], in_=st)

            def outconv(b):
                o3a = big.tile([128, hc, n], F32R, tag="bigC")
                o3b = big.tile([128, cch - hc, n], F32R, tag="bigD")
                nc.sync.dma_start(
                    out=o3a, in_=obuf[b, 0:hc * 128]
                    .rearrange("(ci p) n -> p ci n", p=128))
                nc.sync.dma_start(
                    out=o3b, in_=obuf[b, hc * 128:]
                    .rearrange("(ci p) n -> p ci n", p=128))

                def oci(ci):
                    return o3a[:, ci] if ci < hc else o3b[:, ci - hc]

                for ot in range(cch):
                    wt = wp.tile([128, cch, 128], F32R, tag="w")
                    nc.sync.dma_start(
                        out=wt, in_=woutT[:, ot * 128:(ot + 1) * 128]
                        .rearrange("(ci p) o -> p ci o", p=128))
                    for nh in range(nh2):
                        acc = psA.tile([128, NHALF], F32, tag="acc")
                        for ci in range(cch):
                            nc.tensor.matmul(
                                acc, wt[:, ci],
                                oci(ci)[:, nh * NHALF:(nh + 1) * NHALF],
                                start=(ci == 0), stop=(ci == cch - 1))
                        xr = stp.tile([128, NHALF], F32R, tag="st")
                        nc.sync.dma_start(
                            out=xr,
                            in_=x_in[b, ot * 128:(ot + 1) * 128,
                                     nh * NHALF:(nh + 1) * NHALF])
                        yt = stp.tile([128, NHALF], F32, tag="st")
                        nc.vector.tensor_tensor(
                            out=yt, in0=acc, in1=xr.bitcast(F32),
                            op=mybir.AluOpType.add)
                        nc.sync.dma_start(
                            out=y_out[b, ot * 128:(ot + 1) * 128,
                                      nh * NHALF:(nh + 1) * NHALF], in_=yt)

            for b in range(Bl):
                proj(b)
                for h in range(heads):
                    attn(b, h)
                outconv(b)

    return nc


_CACHE = {}


def _get_nc(Bl, C, n, heads):
    key = (Bl, C, n, heads)
    if key not in _CACHE:
        nc = build_kernel(Bl, C, n, heads)
        if not nc.is_finalized():
            nc.finalize()
        _CACHE[key] = nc
    return _CACHE[key]


def _run(x, w_qkv, w_out, gamma, **spmd_kwargs):
    B, C, H, W = x.shape
    heads = 2
    n = H * W
    Bl = B // NCORES
    xs = np.ascontiguousarray(np.asarray(x, dtype=np.float32).reshape(B, C, n))
    wqkvT = np.ascontiguousarray(np.asarray(w_qkv, dtype=np.float32).T)
    g = np.float32(np.asarray(gamma).reshape(-1)[0])
    woutT = np.ascontiguousarray((g * np.asarray(w_out, dtype=np.float32)).T)

    nc = _get_nc(Bl, C, n, heads)
    in_maps = [
        {"x": np.ascontiguousarray(xs[i * Bl:(i + 1) * Bl]),
         "wqkvT": wqkvT, "woutT": woutT}
        for i in range(NCORES)
    ]
    res = run_bass_kernel_spmd(nc, in_maps, list(range(NCORES)), **spmd_kwargs)
    y = np.concatenate(
        [res.results[i]["y"].reshape(Bl, C, H, W) for i in range(NCORES)], axis=0)
    return y.astype(np.float32), res


def kernel(x, w_qkv, w_out, gamma):
    y, _ = _run(x, w_qkv, w_out, gamma)
    return y

